# revision 9
# baseline (speedup 1.0000x reference)
"""Distributed Trainium2 Bass kernel for one dense transformer block.

Reference computation (B=1, T=2048, D=1024, H=16, HS=64, FF=4096, fp32):
    xn  = rmsnorm(x, g1)
    q,k,v per head; causal softmax attention; sa = attn @ Wproj + bproj
    x   = x + sa
    xn2 = rmsnorm(x, g2)
    x   = x + silu(xn2 @ W1) @ W2

Sharding across 8 NeuronCores:
  - Attention is head-sharded (2 heads/core over the full sequence).
  - Per-head attention keeps keys on the partition axis: sT = kT-block.T @ qT,
    p = exp(sT*scale) (no max subtraction needed -- scores are O(1)), and
    attnT = [v | 1].T @ p accumulated over key tiles, which yields both the
    unnormalized attention output and the softmax denominator in one PSUM
    accumulation chain.  Normalization multiplies by a GpSimd
    partition-broadcast of the reciprocal denominators.
  - Four AllToAlls (one per 512-row query block, overlapping later blocks'
    compute) redistribute attnT from head-sharded to sequence-sharded
    layout.  Core j owns query rows {512*qb + 64*j .. +64} for qb=0..3.
  - proj / residual / rmsnorm2 / FFN run sequence-sharded (256 rows/core)
    with replicated Wproj/W1/W2 streamed from HBM.
  - g1/g2 are folded into Wq/Wk/Wv/W1 on the host; bproj is added via a
    rank-1 matmul into the proj PSUM accumulation.
  - PE-facing tensors are fp16 (10-bit mantissa, ~4e-4 matmul rel err,
    full-rate matmul + fast weight load + half DMA); residual adds and
    softmax/norm statistics stay fp32.

Each core returns its 256 interleaved rows; the host scatters them back.
"""

import numpy as np
from contextlib import ExitStack

import concourse.bass as bass
import concourse.tile as tile
from concourse import bacc, mybir
from concourse import bass_utils

T, D, H, HS, FF = 2048, 1024, 16, 64, 4096
NCORES = 8
HPC = H // NCORES      # heads per core = 2
CH = T // NCORES       # rows per core = 256
QB = 512               # query block
NB = T // QB           # 4 query blocks
ND = D // 128          # 8 contraction tiles
NT = T // 128          # 16 key tiles
SUB = QB // NCORES     # 64 rows per (core, query block)
EPS = 1e-6
SCALE = HS ** -0.5

F32 = mybir.dt.float32
F16 = mybir.dt.float16
AF = mybir.ActivationFunctionType
ALU = mybir.AluOpType

_CACHE = {}


def build_nc():
    if "nc" in _CACHE:
        return _CACHE["nc"]

    nc = bacc.Bacc("TRN2", target_bir_lowering=False, debug=False, num_devices=NCORES)

    xT_d = nc.dram_tensor("xT", [D, T], F16, kind="ExternalInput")
    xch_d = nc.dram_tensor("xch", [CH, D], F32, kind="ExternalInput")
    wq_d = nc.dram_tensor("wq", [D, HPC * HS], F16, kind="ExternalInput")
    wk_d = nc.dram_tensor("wk", [D, HPC * HS], F16, kind="ExternalInput")
    wv_d = nc.dram_tensor("wv", [D, HPC * HS], F16, kind="ExternalInput")
    wp_d = nc.dram_tensor("wp", [D, D], F16, kind="ExternalInput")
    bp_d = nc.dram_tensor("bp", [1, D], F16, kind="ExternalInput")
    w1_d = nc.dram_tensor("w1", [D, FF], F16, kind="ExternalInput")
    w2_d = nc.dram_tensor("w2", [FF, D], F16, kind="ExternalInput")
    ident_d = nc.dram_tensor("ident", [128, 128], F16, kind="ExternalInput")
    ones_c_d = nc.dram_tensor("ones_c", [128, 1], F16, kind="ExternalInput")
    ones_r_d = nc.dram_tensor("ones_r", [1, 128], F16, kind="ExternalInput")
    onescol_d = nc.dram_tensor("onescol", [128, NT], F16, kind="ExternalInput")
    epsb_d = nc.dram_tensor("epsb", [128, 1], F32, kind="ExternalInput")
    out_d = nc.dram_tensor("out", [CH, D], F32, kind="ExternalOutput")

    with tile.TileContext(nc) as tc, ExitStack() as top:
        pers = top.enter_context(tc.tile_pool(name="pers", bufs=1))
        p2 = top.enter_context(tc.tile_pool(name="p2", bufs=2))
        dram = top.enter_context(tc.tile_pool(name="dram", bufs=1, space="DRAM"))

        ident = pers.tile([128, 128], F16, name="ident")
        nc.sync.dma_start(ident[:], ident_d[:])
        ones_c = pers.tile([128, 1], F16, name="ones_c")
        nc.sync.dma_start(ones_c[:], ones_c_d[:])
        ones_r = pers.tile([1, 128], F16, name="ones_r")
        nc.sync.dma_start(ones_r[:], ones_r_d[:])
        bp = pers.tile([1, D], F16, name="bp")
        nc.sync.dma_start(bp[:], bp_d[:])
        epsb = pers.tile([128, 1], F32, name="epsb")
        nc.sync.dma_start(epsb[:], epsb_d[:])

        qT = pers.tile([128, T], F16, name="qT")
        kT = pers.tile([128, T], F16, name="kT")
        va = [pers.tile([128, NT, HS + 1], F16, name=f"va{h}") for h in range(HPC)]
        for h in range(HPC):
            nc.sync.dma_start(va[h][:, :, HS], onescol_d[:])

        wp = [pers.tile([128, D], F16, name=f"wp{f}") for f in range(ND)]
        for f in range(ND):
            nc.sync.dma_start(wp[f][:], wp_d[128 * f : 128 * (f + 1), :])

        bnc_i = [dram.tile([NCORES * 128, SUB], F16, name=f"bi{qb}") for qb in range(NB)]
        bnc_o = [dram.tile([NCORES * 128, SUB], F16, name=f"bo{qb}") for qb in range(NB)]

        # ============ Phase A+B: rmsnorm1 (transposed layout) + QKV^T ============
        with ExitStack() as ph:
            pab = ph.enter_context(tc.tile_pool(name="pab", bufs=1))
            pstr = ph.enter_context(tc.tile_pool(name="pstr", bufs=3))
            psq = ph.enter_context(tc.tile_pool(name="psq", bufs=2, space="PSUM"))
            pss = ph.enter_context(tc.tile_pool(name="pss", bufs=1, space="PSUM"))
            ptp = ph.enter_context(tc.tile_pool(name="ptp", bufs=1, space="PSUM"))

            xt = [pab.tile([128, T], F16, name=f"xt{d}") for d in range(ND)]
            for d in range(ND):
                nc.sync.dma_start(xt[d][:], xT_d[128 * d : 128 * (d + 1), :])
            wq = [pab.tile([128, HPC * HS], F16, name=f"wq{d}") for d in range(ND)]
            wk = [pab.tile([128, HPC * HS], F16, name=f"wk{d}") for d in range(ND)]
            wv = [pab.tile([128, HPC * HS], F16, name=f"wv{d}") for d in range(ND)]
            for d in range(ND):
                sl = slice(128 * d, 128 * (d + 1))
                nc.sync.dma_start(wq[d][:], wq_d[sl, :])
                nc.sync.dma_start(wk[d][:], wk_d[sl, :])
                nc.sync.dma_start(wv[d][:], wv_d[sl, :])

            for tb in range(NB):
                cs = slice(QB * tb, QB * (tb + 1))
                # raw QKV^T on unnormalized x; rstd is folded in at evacuation
                ps_q = psq.tile([128, QB], F32, name="ps_q")
                ps_k = psq.tile([128, QB], F32, name="ps_k")
                ps_v = psq.tile([128, QB], F32, name="ps_v")
                for d in range(ND):
                    st, sp = (d == 0), (d == ND - 1)
                    nc.tensor.matmul(ps_q[:], wq[d][:], xt[d][:, cs], start=st, stop=sp)
                    nc.tensor.matmul(ps_k[:], wk[d][:], xt[d][:, cs], start=st, stop=sp)
                    nc.tensor.matmul(ps_v[:], wv[d][:], xt[d][:, cs], start=st, stop=sp)

                # rmsnorm stats in parallel with the QKV matmuls
                ps_ss = pss.tile([1, QB], F32, name="ps_ss")
                for d in range(ND):
                    sq = pstr.tile([128, QB], F16, name="sq")
                    nc.gpsimd.tensor_mul(sq[:], xt[d][:, cs], xt[d][:, cs])
                    nc.tensor.matmul(
                        ps_ss[:], ones_c[:], sq[:], start=(d == 0), stop=(d == ND - 1)
                    )
                sqr = pstr.tile([1, QB], F32, name="sqr")
                nc.scalar.activation(
                    sqr[:], ps_ss[:], AF.Sqrt, scale=1.0 / D, bias=epsb[0:1, :]
                )
                rstd = pstr.tile([1, QB], F16, name="rstd")
                with nc.allow_low_precision("fp16 rounding is intended"):
                    nc.vector.reciprocal(rstd[:], sqr[:])
                bc = pstr.tile([128, QB], F16, name="bc")
                nc.gpsimd.partition_broadcast(bc[:], rstd[:])

                nc.vector.tensor_mul(qT[:, cs], ps_q[:], bc[:])
                nc.vector.tensor_mul(kT[:, cs], ps_k[:], bc[:])
                vt = pstr.tile([128, QB], F16, name="vt")
                nc.vector.tensor_mul(vt[:], ps_v[:], bc[:])
                for s in range(4):
                    tt = 4 * tb + s
                    ps_t = ptp.tile([128, 128], F16, name="ps_t")
                    nc.tensor.transpose(
                        ps_t[:], vt[:, 128 * s : 128 * (s + 1)], ident[:]
                    )
                    for h in range(HPC):
                        nc.vector.tensor_copy(
                            va[h][:, tt, 0:HS], ps_t[:, HS * h : HS * (h + 1)]
                        )

        # ============ Phase C: causal attention + per-block AllToAll =============
        with ExitStack() as ph:
            ppt = ph.enter_context(tc.tile_pool(name="ppt", bufs=4))
            psat = ph.enter_context(tc.tile_pool(name="psat", bufs=2, space="PSUM"))
            pscs = ph.enter_context(tc.tile_pool(name="pscs", bufs=2, space="PSUM"))

            for qb in range(NB):
                qs = slice(QB * qb, QB * (qb + 1))
                nkt = 4 * (qb + 1)
                ps_at = [
                    psat.tile([HS + 1, QB], F32, name=f"at{h}", tag=f"at{h}")
                    for h in range(HPC)
                ]
                for kt0 in range(0, nkt, 2):
                    for h in range(HPC):
                        hsl = slice(HS * h, HS * (h + 1))
                        ps_s = pscs.tile([128, 1024], F32, name="ps_s", tag="ps_s")
                        for i in range(2):
                            kt = kt0 + i
                            nc.tensor.matmul(
                                ps_s[:, 512 * i : 512 * (i + 1)],
                                kT[hsl, 128 * kt : 128 * (kt + 1)],
                                qT[hsl, qs],
                                start=True,
                                stop=True,
                            )
                        pt = ppt.tile([128, 1024], F16, name="pt")
                        nc.scalar.activation(pt[:], ps_s[:], AF.Exp, scale=SCALE)
                        for i in range(2):
                            kt = kt0 + i
                            if kt >= 4 * qb:  # diagonal tile: zero where k > q
                                nc.gpsimd.affine_select(
                                    pt[:, 512 * i : 512 * (i + 1)],
                                    pt[:, 512 * i : 512 * (i + 1)],
                                    pattern=[[1, 512]],
                                    compare_op=ALU.is_ge,
                                    fill=0.0,
                                    base=QB * qb - 128 * kt,
                                    channel_multiplier=-1,
                                )
                        for i in range(2):
                            kt = kt0 + i
                            nc.tensor.matmul(
                                ps_at[h][:],
                                va[h][:, kt, :],
                                pt[:, 512 * i : 512 * (i + 1)],
                                start=(kt == 0),
                                stop=(kt == nkt - 1),
                            )
                attnT2 = p2.tile([128, QB], F16, name="attnT2")
                for h in range(HPC):
                    hsl = slice(HS * h, HS * (h + 1))
                    recip_h = p2.tile([1, QB], F16, name="recip", tag="recip")
                    with nc.allow_low_precision("fp16 rounding is intended"):
                        nc.vector.reciprocal(recip_h[:], ps_at[h][HS : HS + 1, :])
                    bc_sb = p2.tile([HS, QB], F16, name="bc_sb", tag="bc_sb")
                    nc.gpsimd.partition_broadcast(bc_sb[:], recip_h[:])
                    nc.vector.tensor_mul(attnT2[hsl, :], ps_at[h][0:HS, :], bc_sb[:])
                for j in range(NCORES):
                    nc.scalar.dma_start(
                        bnc_i[qb][128 * j : 128 * (j + 1), :],
                        attnT2[:, SUB * j : SUB * (j + 1)],
                    )
                nc.gpsimd.collective_compute(
                    "AllToAll",
                    ALU.bypass,
                    replica_groups=[list(range(NCORES))],
                    ins=[bnc_i[qb].opt()],
                    outs=[bnc_o[qb].opt()],
                )

        # ============ Phase D: proj + residual + rmsnorm2 (+ transpose) ==========
        x2 = [pers.tile([128, D], F32, name=f"x2_{ts}") for ts in range(2)]
        xn2T = [pers.tile([128, CH], F16, name=f"xn2T{d}") for d in range(ND)]
        with ExitStack() as ph:
            pd = ph.enter_context(tc.tile_pool(name="pd", bufs=1))
            pds = ph.enter_context(tc.tile_pool(name="pds", bufs=2))
            psd1 = ph.enter_context(tc.tile_pool(name="psd1", bufs=2, space="PSUM"))
            psd2 = ph.enter_context(tc.tile_pool(name="psd2", bufs=2, space="PSUM"))

            aT = [pd.tile([128, CH], F16, name=f"aT{f}") for f in range(ND)]
            for qb in range(NB):
                for f in range(ND):
                    nc.scalar.dma_start(
                        aT[f][:, SUB * qb : SUB * (qb + 1)],
                        bnc_o[qb][128 * f : 128 * (f + 1), :],
                    )
            xch = [pd.tile([128, D], F32, name=f"xch{ts}") for ts in range(2)]
            for ts in range(2):
                nc.scalar.dma_start(xch[ts][:], xch_d[128 * ts : 128 * (ts + 1), :])

            for ts in range(2):
                tsl = slice(128 * ts, 128 * (ts + 1))
                ps_sa = psd1.tile([128, D], F32, name="ps_sa")
                for b in range(2):
                    bsl = slice(512 * b, 512 * (b + 1))
                    nc.tensor.matmul(
                        ps_sa[:, bsl], ones_r[:], bp[0:1, bsl], start=True, stop=False
                    )
                    for f in range(ND):
                        nc.tensor.matmul(
                            ps_sa[:, bsl],
                            aT[f][:, tsl],
                            wp[f][:, bsl],
                            start=False,
                            stop=(f == ND - 1),
                        )
                nc.vector.tensor_add(x2[ts][:], ps_sa[:], xch[ts][:])
                sq2 = pds.tile([128, D], F32, name="sq2")
                ss2 = pds.tile([128, 1], F32, name="ss2")
                nc.scalar.activation(sq2[:], x2[ts][:], AF.Square, accum_out=ss2[:])
                sqr2 = pds.tile([128, 1], F32, name="sqr2")
                nc.scalar.activation(
                    sqr2[:], ss2[:], AF.Sqrt, scale=1.0 / D, bias=epsb[:]
                )
                rstd2 = pds.tile([128, 1], F32, name="rstd2")
                nc.vector.reciprocal(rstd2[:], sqr2[:])
                xn2 = pds.tile([128, D], F16, name="xn2")
                with nc.allow_low_precision("fp16 rounding is intended"):
                    nc.scalar.activation(xn2[:], x2[ts][:], AF.Copy, scale=rstd2[:])
                for d in range(ND):
                    ps_t2 = psd2.tile([128, 128], F16, name="ps_t2")
                    nc.tensor.transpose(
                        ps_t2[:], xn2[:, 128 * d : 128 * (d + 1)], ident[:]
                    )
                    nc.vector.tensor_copy(xn2T[d][:, tsl], ps_t2[:])

        # ============ Phase E: FFN (replicated fp16 weights, 256 rows) ===========
        with ExitStack() as ph:
            pw1 = ph.enter_context(tc.tile_pool(name="pw1", bufs=16))
            pw2 = ph.enter_context(tc.tile_pool(name="pw2", bufs=6))
            pht = ph.enter_context(tc.tile_pool(name="pht", bufs=4))
            pse1 = ph.enter_context(tc.tile_pool(name="pse1", bufs=1, space="PSUM"))
            pse3 = ph.enter_context(tc.tile_pool(name="pse3", bufs=3, space="PSUM"))

            ps_out = [pse1.tile([128, D], F32, name=f"ps_out{ts}") for ts in range(2)]
            NG = 8
            for g in range(NG):
                gsl = slice(512 * g, 512 * (g + 1))
                w1g = []
                for d in range(ND):
                    t_ = pw1.tile([128, 512], F16, name="w1t", tag="w1t")
                    nc.sync.dma_start(t_[:], w1_d[128 * d : 128 * (d + 1), gsl])
                    w1g.append(t_)
                for j in range(4):
                    ff = 4 * g + j
                    ps_h = pse3.tile([128, CH], F32, name="ps_h")
                    for d in range(ND):
                        nc.tensor.matmul(
                            ps_h[:],
                            w1g[d][:, 128 * j : 128 * (j + 1)],
                            xn2T[d][:],
                            start=(d == 0),
                            stop=(d == ND - 1),
                        )
                    hT = pht.tile([128, CH], F16, name="hT", tag="hT")
                    nc.scalar.activation(hT[:], ps_h[:], AF.Silu)
                    w2t = pw2.tile([128, D], F16, name="w2t", tag="w2t")
                    nc.sync.dma_start(w2t[:], w2_d[128 * ff : 128 * (ff + 1), :])
                    for ts in range(2):
                        for b in range(2):
                            nc.tensor.matmul(
                                ps_out[ts][:, 512 * b : 512 * (b + 1)],
                                hT[:, 128 * ts : 128 * (ts + 1)],
                                w2t[:, 512 * b : 512 * (b + 1)],
                                start=(ff == 0),
                                stop=(ff == FF // 128 - 1),
                            )
            for ts in range(2):
                out_t = pht.tile([128, D], F32, name=f"out{ts}", tag=f"out{ts}")
                nc.vector.tensor_add(out_t[:], ps_out[ts][:], x2[ts][:])
                nc.scalar.dma_start(out_d[128 * ts : 128 * (ts + 1), :], out_t[:])

    nc.compile()
    _CACHE["nc"] = nc
    return nc


def _chunk_rows(j):
    """Row indices owned by core j (64 rows out of each 512-row block)."""
    return np.concatenate(
        [np.arange(QB * qb + SUB * j, QB * qb + SUB * j + SUB) for qb in range(NB)]
    )


def make_in_maps(inputs):
    x = np.asarray(inputs["x"], np.float32).reshape(T, D)
    Wq = np.asarray(inputs["Wq"], np.float32)
    Wk = np.asarray(inputs["Wk"], np.float32)
    Wv = np.asarray(inputs["Wv"], np.float32)
    Wproj = np.asarray(inputs["Wproj"], np.float32)
    bproj = np.asarray(inputs["bproj"], np.float32).reshape(1, D)
    W1 = np.asarray(inputs["W1"], np.float32)
    W2 = np.asarray(inputs["W2"], np.float32)
    g1 = np.asarray(inputs["g1"], np.float32)
    g2 = np.asarray(inputs["g2"], np.float32)

    xT = np.ascontiguousarray(x.T).astype(np.float16)
    Wq_f = (Wq * g1[None, :, None]).astype(np.float16)
    Wk_f = (Wk * g1[None, :, None]).astype(np.float16)
    Wv_f = (Wv * g1[None, :, None]).astype(np.float16)
    W1_f = np.ascontiguousarray(W1 * g2[:, None]).astype(np.float16)
    W2_h = np.ascontiguousarray(W2.astype(np.float16))
    Wp_h = np.ascontiguousarray(Wproj.astype(np.float16))

    common = {
        "xT": xT,
        "wp": Wp_h,
        "bp": bproj.astype(np.float16),
        "w1": W1_f,
        "w2": W2_h,
        "ident": np.eye(128, dtype=np.float16),
        "ones_c": np.ones((128, 1), np.float16),
        "ones_r": np.ones((1, 128), np.float16),
        "onescol": np.ones((128, NT), np.float16),
        "epsb": np.full((128, 1), EPS, np.float32),
    }
    in_maps = []
    for c in range(NCORES):
        heads = [HPC * c + h for h in range(HPC)]
        in_maps.append(
            {
                **common,
                "xch": np.ascontiguousarray(x[_chunk_rows(c)]),
                "wq": np.ascontiguousarray(np.concatenate([Wq_f[h] for h in heads], 1)),
                "wk": np.ascontiguousarray(np.concatenate([Wk_f[h] for h in heads], 1)),
                "wv": np.ascontiguousarray(np.concatenate([Wv_f[h] for h in heads], 1)),
            }
        )
    return in_maps


def run(inputs, **kwargs):
    nc = build_nc()
    in_maps = make_in_maps(inputs)
    res = bass_utils.run_bass_kernel_spmd(
        nc, in_maps, core_ids=list(range(NCORES)), **kwargs
    )
    out = np.empty((T, D), np.float32)
    for c in range(NCORES):
        out[_chunk_rows(c)] = res.results[c]["out"]
    return out.reshape(1, T, D), res


def kernel(**inputs):
    out, _ = run(inputs)
    return out


# revision 11
# speedup vs baseline: 1.0788x; 1.0788x over previous
"""Distributed Trainium2 Bass kernel for one dense transformer block.

Reference computation (B=1, T=2048, D=1024, H=16, HS=64, FF=4096, fp32):
    xn  = rmsnorm(x, g1)
    q,k,v per head; causal softmax attention; sa = attn @ Wproj + bproj
    x   = x + sa
    xn2 = rmsnorm(x, g2)
    x   = x + silu(xn2 @ W1) @ W2

Sharding across 8 NeuronCores:
  - Attention is head-sharded (2 heads/core over the full sequence).
  - Per-head attention keeps keys on the partition axis: sT = kT-block.T @ qT,
    p = exp(sT*scale) (no max subtraction needed -- scores are O(1)), and
    attnT = [v | 1].T @ p accumulated over key tiles, which yields both the
    unnormalized attention output and the softmax denominator in one PSUM
    accumulation chain.  Normalization multiplies by a GpSimd
    partition-broadcast of the reciprocal denominators.
  - Four AllToAlls (one per 512-row query block, overlapping later blocks'
    compute) redistribute attnT from head-sharded to sequence-sharded
    layout.  Core j owns query rows {512*qb + 64*j .. +64} for qb=0..3.
  - proj / residual / rmsnorm2 / FFN run sequence-sharded (256 rows/core)
    with replicated Wproj/W1/W2 streamed from HBM.
  - g1/g2 are folded into Wq/Wk/Wv/W1 on the host; bproj is added via a
    rank-1 matmul into the proj PSUM accumulation.
  - PE-facing tensors are fp16 (10-bit mantissa, ~4e-4 matmul rel err,
    full-rate matmul + fast weight load + half DMA); residual adds and
    softmax/norm statistics stay fp32.

Each core returns its 256 interleaved rows; the host scatters them back.
"""

import numpy as np
from contextlib import ExitStack

import concourse.bass as bass
import concourse.tile as tile
from concourse import bacc, mybir
from concourse import bass_utils

T, D, H, HS, FF = 2048, 1024, 16, 64, 4096
NCORES = 8
HPC = H // NCORES      # heads per core = 2
CH = T // NCORES       # rows per core = 256
QB = 512               # query block
NB = T // QB           # 4 query blocks
ND = D // 128          # 8 contraction tiles
NT = T // 128          # 16 key tiles
SUB = QB // NCORES     # 64 rows per (core, query block)
EPS = 1e-6
SCALE = HS ** -0.5

F32 = mybir.dt.float32
F16 = mybir.dt.float16
AF = mybir.ActivationFunctionType
ALU = mybir.AluOpType

_CACHE = {}


def build_nc():
    if "nc" in _CACHE:
        return _CACHE["nc"]

    nc = bacc.Bacc("TRN2", target_bir_lowering=False, debug=False, num_devices=NCORES)

    xT_d = nc.dram_tensor("xT", [D, T], F16, kind="ExternalInput")
    xch_d = nc.dram_tensor("xch", [CH, D], F32, kind="ExternalInput")
    wq_d = nc.dram_tensor("wq", [D, HPC * HS], F16, kind="ExternalInput")
    wk_d = nc.dram_tensor("wk", [D, HPC * HS], F16, kind="ExternalInput")
    wv_d = nc.dram_tensor("wv", [D, HPC * HS], F16, kind="ExternalInput")
    wp_d = nc.dram_tensor("wp", [D, D], F16, kind="ExternalInput")
    bp_d = nc.dram_tensor("bp", [1, D], F16, kind="ExternalInput")
    w1_d = nc.dram_tensor("w1", [D, FF], F16, kind="ExternalInput")
    w2_d = nc.dram_tensor("w2", [FF, D], F16, kind="ExternalInput")
    ident_d = nc.dram_tensor("ident", [128, 128], F16, kind="ExternalInput")
    ones_c_d = nc.dram_tensor("ones_c", [128, 1], F16, kind="ExternalInput")
    ones_r_d = nc.dram_tensor("ones_r", [1, 128], F16, kind="ExternalInput")
    onescol_d = nc.dram_tensor("onescol", [128, NT], F16, kind="ExternalInput")
    epsb_d = nc.dram_tensor("epsb", [128, 1], F32, kind="ExternalInput")
    out_d = nc.dram_tensor("out", [CH, D], F32, kind="ExternalOutput")

    with tile.TileContext(nc) as tc, ExitStack() as top:
        pers = top.enter_context(tc.tile_pool(name="pers", bufs=1))
        p2 = top.enter_context(tc.tile_pool(name="p2", bufs=2))
        dram = top.enter_context(tc.tile_pool(name="dram", bufs=1, space="DRAM"))

        ident = pers.tile([128, 128], F16, name="ident")
        nc.sync.dma_start(ident[:], ident_d[:])
        ones_c = pers.tile([128, 1], F16, name="ones_c")
        nc.sync.dma_start(ones_c[:], ones_c_d[:])
        ones_r = pers.tile([1, 128], F16, name="ones_r")
        nc.sync.dma_start(ones_r[:], ones_r_d[:])
        bp = pers.tile([1, D], F16, name="bp")
        nc.sync.dma_start(bp[:], bp_d[:])
        epsb = pers.tile([128, 1], F32, name="epsb")
        nc.sync.dma_start(epsb[:], epsb_d[:])

        qT = pers.tile([128, T], F16, name="qT")
        kT = pers.tile([128, T], F16, name="kT")
        va = [pers.tile([128, NT, HS + 1], F16, name=f"va{h}") for h in range(HPC)]
        for h in range(HPC):
            nc.sync.dma_start(va[h][:, :, HS], onescol_d[:])

        wp = [pers.tile([128, D], F16, name=f"wp{f}") for f in range(ND)]
        for f in range(ND):
            nc.sync.dma_start(wp[f][:], wp_d[128 * f : 128 * (f + 1), :])

        bnc_i = [dram.tile([NCORES * 128, SUB], F16, name=f"bi{qb}") for qb in range(NB)]
        bnc_o = [dram.tile([NCORES * 128, SUB], F16, name=f"bo{qb}") for qb in range(NB)]

        # ============ Phase A+B: rmsnorm1 (transposed layout) + QKV^T ============
        with ExitStack() as ph:
            pab = ph.enter_context(tc.tile_pool(name="pab", bufs=1))
            pstr = ph.enter_context(tc.tile_pool(name="pstr", bufs=3))
            psq = ph.enter_context(tc.tile_pool(name="psq", bufs=2, space="PSUM"))
            pss = ph.enter_context(tc.tile_pool(name="pss", bufs=1, space="PSUM"))
            ptp = ph.enter_context(tc.tile_pool(name="ptp", bufs=1, space="PSUM"))

            xt = [pab.tile([128, T], F16, name=f"xt{d}") for d in range(ND)]
            for d in range(ND):
                nc.scalar.dma_start(xt[d][:], xT_d[128 * d : 128 * (d + 1), :])
            wq = [pab.tile([128, HPC * HS], F16, name=f"wq{d}") for d in range(ND)]
            wk = [pab.tile([128, HPC * HS], F16, name=f"wk{d}") for d in range(ND)]
            wv = [pab.tile([128, HPC * HS], F16, name=f"wv{d}") for d in range(ND)]
            for d in range(ND):
                sl = slice(128 * d, 128 * (d + 1))
                nc.sync.dma_start(wq[d][:], wq_d[sl, :])
                nc.sync.dma_start(wk[d][:], wk_d[sl, :])
                nc.sync.dma_start(wv[d][:], wv_d[sl, :])

            # rmsnorm stats for every block first: squares on DVE, partition-sum
            # on PE, so nothing blocks the later dense QKV stream.
            bcs = []
            for tb in range(NB):
                cs = slice(QB * tb, QB * (tb + 1))
                ps_ss = pss.tile([1, QB], F32, name="ps_ss", tag="ps_ss")
                for d in range(ND):
                    sq = pstr.tile([128, QB], F16, name="sq")
                    nc.vector.tensor_mul(sq[:], xt[d][:, cs], xt[d][:, cs])
                    nc.tensor.matmul(
                        ps_ss[:], ones_c[:], sq[:], start=(d == 0), stop=(d == ND - 1)
                    )
                sqr = pstr.tile([1, QB], F32, name="sqr")
                nc.scalar.activation(
                    sqr[:], ps_ss[:], AF.Sqrt, scale=1.0 / D, bias=epsb[0:1, :]
                )
                rstd = pstr.tile([1, QB], F16, name="rstd")
                with nc.allow_low_precision("fp16 rounding is intended"):
                    nc.vector.reciprocal(rstd[:], sqr[:])
                bc = pab.tile([128, QB], F16, name=f"bc{tb}")
                nc.gpsimd.partition_broadcast(bc[:], rstd[:])
                bcs.append(bc)

            # raw QKV^T on unnormalized x; rstd is folded in at evacuation
            for tb in range(NB):
                cs = slice(QB * tb, QB * (tb + 1))
                bc = bcs[tb]
                ps_q = psq.tile([128, QB], F32, name="ps_q")
                ps_k = psq.tile([128, QB], F32, name="ps_k")
                ps_v = psq.tile([128, QB], F32, name="ps_v")
                for d in range(ND):
                    st, sp = (d == 0), (d == ND - 1)
                    nc.tensor.matmul(ps_q[:], wq[d][:], xt[d][:, cs], start=st, stop=sp)
                    nc.tensor.matmul(ps_k[:], wk[d][:], xt[d][:, cs], start=st, stop=sp)
                    nc.tensor.matmul(ps_v[:], wv[d][:], xt[d][:, cs], start=st, stop=sp)
                nc.vector.tensor_mul(qT[:, cs], ps_q[:], bc[:])
                nc.vector.tensor_mul(kT[:, cs], ps_k[:], bc[:])
                vt = pstr.tile([128, QB], F16, name="vt")
                nc.vector.tensor_mul(vt[:], ps_v[:], bc[:])
                for s in range(4):
                    tt = 4 * tb + s
                    ps_t = ptp.tile([128, 128], F16, name="ps_t")
                    nc.tensor.transpose(
                        ps_t[:], vt[:, 128 * s : 128 * (s + 1)], ident[:]
                    )
                    for h in range(HPC):
                        nc.vector.tensor_copy(
                            va[h][:, tt, 0:HS], ps_t[:, HS * h : HS * (h + 1)]
                        )

        # ============ Phase C: causal attention + per-block AllToAll =============
        with ExitStack() as ph:
            ppt = ph.enter_context(tc.tile_pool(name="ppt", bufs=4))
            psat = ph.enter_context(tc.tile_pool(name="psat", bufs=2, space="PSUM"))
            pscs = ph.enter_context(tc.tile_pool(name="pscs", bufs=2, space="PSUM"))

            for qb in range(NB):
                qs = slice(QB * qb, QB * (qb + 1))
                nkt = 4 * (qb + 1)
                ps_at = [
                    psat.tile([HS + 1, QB], F32, name=f"at{h}", tag=f"at{h}")
                    for h in range(HPC)
                ]
                for kt0 in range(0, nkt, 2):
                    for h in range(HPC):
                        hsl = slice(HS * h, HS * (h + 1))
                        ps_s = pscs.tile([128, 1024], F32, name="ps_s", tag="ps_s")
                        for i in range(2):
                            kt = kt0 + i
                            nc.tensor.matmul(
                                ps_s[:, 512 * i : 512 * (i + 1)],
                                kT[hsl, 128 * kt : 128 * (kt + 1)],
                                qT[hsl, qs],
                                start=True,
                                stop=True,
                            )
                        pt = ppt.tile([128, 1024], F16, name="pt")
                        nc.scalar.activation(pt[:], ps_s[:], AF.Exp, scale=SCALE)
                        for i in range(2):
                            kt = kt0 + i
                            if kt >= 4 * qb:  # diagonal tile: zero where k > q
                                nc.gpsimd.affine_select(
                                    pt[:, 512 * i : 512 * (i + 1)],
                                    pt[:, 512 * i : 512 * (i + 1)],
                                    pattern=[[1, 512]],
                                    compare_op=ALU.is_ge,
                                    fill=0.0,
                                    base=QB * qb - 128 * kt,
                                    channel_multiplier=-1,
                                )
                        for i in range(2):
                            kt = kt0 + i
                            nc.tensor.matmul(
                                ps_at[h][:],
                                va[h][:, kt, :],
                                pt[:, 512 * i : 512 * (i + 1)],
                                start=(kt == 0),
                                stop=(kt == nkt - 1),
                            )
                attnT2 = p2.tile([128, QB], F16, name="attnT2")
                for h in range(HPC):
                    hsl = slice(HS * h, HS * (h + 1))
                    recip_h = p2.tile([1, QB], F16, name="recip", tag="recip")
                    with nc.allow_low_precision("fp16 rounding is intended"):
                        nc.vector.reciprocal(recip_h[:], ps_at[h][HS : HS + 1, :])
                    bc_sb = p2.tile([HS, QB], F16, name="bc_sb", tag="bc_sb")
                    nc.gpsimd.partition_broadcast(bc_sb[:], recip_h[:])
                    nc.vector.tensor_mul(attnT2[hsl, :], ps_at[h][0:HS, :], bc_sb[:])
                for j in range(NCORES):
                    nc.scalar.dma_start(
                        bnc_i[qb][128 * j : 128 * (j + 1), :],
                        attnT2[:, SUB * j : SUB * (j + 1)],
                    )
                nc.gpsimd.collective_compute(
                    "AllToAll",
                    ALU.bypass,
                    replica_groups=[list(range(NCORES))],
                    ins=[bnc_i[qb].opt()],
                    outs=[bnc_o[qb].opt()],
                )

        # ============ Phase D: proj + residual + rmsnorm2 (+ transpose) ==========
        x2 = [pers.tile([128, D], F32, name=f"x2_{ts}") for ts in range(2)]
        xn2T = [pers.tile([128, CH], F16, name=f"xn2T{d}") for d in range(ND)]
        with ExitStack() as ph:
            pd = ph.enter_context(tc.tile_pool(name="pd", bufs=1))
            pds = ph.enter_context(tc.tile_pool(name="pds", bufs=2))
            psd1 = ph.enter_context(tc.tile_pool(name="psd1", bufs=2, space="PSUM"))
            psd2 = ph.enter_context(tc.tile_pool(name="psd2", bufs=2, space="PSUM"))

            aT = [pd.tile([128, CH], F16, name=f"aT{f}") for f in range(ND)]
            for qb in range(NB):
                for f in range(ND):
                    nc.scalar.dma_start(
                        aT[f][:, SUB * qb : SUB * (qb + 1)],
                        bnc_o[qb][128 * f : 128 * (f + 1), :],
                    )
            xch = [pd.tile([128, D], F32, name=f"xch{ts}") for ts in range(2)]
            for ts in range(2):
                nc.scalar.dma_start(xch[ts][:], xch_d[128 * ts : 128 * (ts + 1), :])

            for ts in range(2):
                tsl = slice(128 * ts, 128 * (ts + 1))
                ps_sa = psd1.tile([128, D], F32, name="ps_sa")
                for b in range(2):
                    bsl = slice(512 * b, 512 * (b + 1))
                    nc.tensor.matmul(
                        ps_sa[:, bsl], ones_r[:], bp[0:1, bsl], start=True, stop=False
                    )
                    for f in range(ND):
                        nc.tensor.matmul(
                            ps_sa[:, bsl],
                            aT[f][:, tsl],
                            wp[f][:, bsl],
                            start=False,
                            stop=(f == ND - 1),
                        )
                nc.vector.tensor_add(x2[ts][:], ps_sa[:], xch[ts][:])
                sq2 = pds.tile([128, D], F32, name="sq2")
                ss2 = pds.tile([128, 1], F32, name="ss2")
                nc.scalar.activation(sq2[:], x2[ts][:], AF.Square, accum_out=ss2[:])
                sqr2 = pds.tile([128, 1], F32, name="sqr2")
                nc.scalar.activation(
                    sqr2[:], ss2[:], AF.Sqrt, scale=1.0 / D, bias=epsb[:]
                )
                rstd2 = pds.tile([128, 1], F32, name="rstd2")
                nc.vector.reciprocal(rstd2[:], sqr2[:])
                xn2 = pds.tile([128, D], F16, name="xn2")
                with nc.allow_low_precision("fp16 rounding is intended"):
                    nc.scalar.activation(xn2[:], x2[ts][:], AF.Copy, scale=rstd2[:])
                for d in range(ND):
                    ps_t2 = psd2.tile([128, 128], F16, name="ps_t2")
                    nc.tensor.transpose(
                        ps_t2[:], xn2[:, 128 * d : 128 * (d + 1)], ident[:]
                    )
                    nc.vector.tensor_copy(xn2T[d][:, tsl], ps_t2[:])

        # ============ Phase E: FFN (replicated fp16 weights, 256 rows) ===========
        with ExitStack() as ph:
            pw1 = ph.enter_context(tc.tile_pool(name="pw1", bufs=16))
            pw2 = ph.enter_context(tc.tile_pool(name="pw2", bufs=6))
            pht = ph.enter_context(tc.tile_pool(name="pht", bufs=4))
            pse1 = ph.enter_context(tc.tile_pool(name="pse1", bufs=1, space="PSUM"))
            pse3 = ph.enter_context(tc.tile_pool(name="pse3", bufs=3, space="PSUM"))

            ps_out = [pse1.tile([128, D], F32, name=f"ps_out{ts}") for ts in range(2)]
            NG = 8
            for g in range(NG):
                gsl = slice(512 * g, 512 * (g + 1))
                w1g = []
                for d in range(ND):
                    t_ = pw1.tile([128, 512], F16, name="w1t", tag="w1t")
                    nc.sync.dma_start(t_[:], w1_d[128 * d : 128 * (d + 1), gsl])
                    w1g.append(t_)
                for j in range(4):
                    ff = 4 * g + j
                    ps_h = pse3.tile([128, CH], F32, name="ps_h")
                    for d in range(ND):
                        nc.tensor.matmul(
                            ps_h[:],
                            w1g[d][:, 128 * j : 128 * (j + 1)],
                            xn2T[d][:],
                            start=(d == 0),
                            stop=(d == ND - 1),
                        )
                    hT = pht.tile([128, CH], F16, name="hT", tag="hT")
                    nc.scalar.activation(hT[:], ps_h[:], AF.Silu)
                    w2t = pw2.tile([128, D], F16, name="w2t", tag="w2t")
                    nc.sync.dma_start(w2t[:], w2_d[128 * ff : 128 * (ff + 1), :])
                    for ts in range(2):
                        for b in range(2):
                            nc.tensor.matmul(
                                ps_out[ts][:, 512 * b : 512 * (b + 1)],
                                hT[:, 128 * ts : 128 * (ts + 1)],
                                w2t[:, 512 * b : 512 * (b + 1)],
                                start=(ff == 0),
                                stop=(ff == FF // 128 - 1),
                            )
            for ts in range(2):
                out_t = pht.tile([128, D], F32, name=f"out{ts}", tag=f"out{ts}")
                nc.vector.tensor_add(out_t[:], ps_out[ts][:], x2[ts][:])
                nc.scalar.dma_start(out_d[128 * ts : 128 * (ts + 1), :], out_t[:])

    nc.compile()
    _CACHE["nc"] = nc
    return nc


def _chunk_rows(j):
    """Row indices owned by core j (64 rows out of each 512-row block)."""
    return np.concatenate(
        [np.arange(QB * qb + SUB * j, QB * qb + SUB * j + SUB) for qb in range(NB)]
    )


def make_in_maps(inputs):
    x = np.asarray(inputs["x"], np.float32).reshape(T, D)
    Wq = np.asarray(inputs["Wq"], np.float32)
    Wk = np.asarray(inputs["Wk"], np.float32)
    Wv = np.asarray(inputs["Wv"], np.float32)
    Wproj = np.asarray(inputs["Wproj"], np.float32)
    bproj = np.asarray(inputs["bproj"], np.float32).reshape(1, D)
    W1 = np.asarray(inputs["W1"], np.float32)
    W2 = np.asarray(inputs["W2"], np.float32)
    g1 = np.asarray(inputs["g1"], np.float32)
    g2 = np.asarray(inputs["g2"], np.float32)

    xT = np.ascontiguousarray(x.T).astype(np.float16)
    Wq_f = (Wq * g1[None, :, None]).astype(np.float16)
    Wk_f = (Wk * g1[None, :, None]).astype(np.float16)
    Wv_f = (Wv * g1[None, :, None]).astype(np.float16)
    W1_f = np.ascontiguousarray(W1 * g2[:, None]).astype(np.float16)
    W2_h = np.ascontiguousarray(W2.astype(np.float16))
    Wp_h = np.ascontiguousarray(Wproj.astype(np.float16))

    common = {
        "xT": xT,
        "wp": Wp_h,
        "bp": bproj.astype(np.float16),
        "w1": W1_f,
        "w2": W2_h,
        "ident": np.eye(128, dtype=np.float16),
        "ones_c": np.ones((128, 1), np.float16),
        "ones_r": np.ones((1, 128), np.float16),
        "onescol": np.ones((128, NT), np.float16),
        "epsb": np.full((128, 1), EPS, np.float32),
    }
    in_maps = []
    for c in range(NCORES):
        heads = [HPC * c + h for h in range(HPC)]
        in_maps.append(
            {
                **common,
                "xch": np.ascontiguousarray(x[_chunk_rows(c)]),
                "wq": np.ascontiguousarray(np.concatenate([Wq_f[h] for h in heads], 1)),
                "wk": np.ascontiguousarray(np.concatenate([Wk_f[h] for h in heads], 1)),
                "wv": np.ascontiguousarray(np.concatenate([Wv_f[h] for h in heads], 1)),
            }
        )
    return in_maps


def run(inputs, **kwargs):
    nc = build_nc()
    in_maps = make_in_maps(inputs)
    res = bass_utils.run_bass_kernel_spmd(
        nc, in_maps, core_ids=list(range(NCORES)), **kwargs
    )
    out = np.empty((T, D), np.float32)
    for c in range(NCORES):
        out[_chunk_rows(c)] = res.results[c]["out"]
    return out.reshape(1, T, D), res


def kernel(**inputs):
    out, _ = run(inputs)
    return out


# revision 12
# speedup vs baseline: 1.1367x; 1.0537x over previous
"""Distributed Trainium2 Bass kernel for one dense transformer block.

Reference computation (B=1, T=2048, D=1024, H=16, HS=64, FF=4096, fp32):
    xn  = rmsnorm(x, g1)
    q,k,v per head; causal softmax attention; sa = attn @ Wproj + bproj
    x   = x + sa
    xn2 = rmsnorm(x, g2)
    x   = x + silu(xn2 @ W1) @ W2

Sharding across 8 NeuronCores:
  - Attention is head-sharded (2 heads/core over the full sequence).
  - Per-head attention keeps keys on the partition axis: sT = kT-block.T @ qT,
    p = exp(sT*scale) (no max subtraction needed -- scores are O(1)), and
    attnT = [v | 1].T @ p accumulated over key tiles, which yields both the
    unnormalized attention output and the softmax denominator in one PSUM
    accumulation chain.  Normalization multiplies by a GpSimd
    partition-broadcast of the reciprocal denominators.
  - Four AllToAlls (one per 512-row query block, overlapping later blocks'
    compute) redistribute attnT from head-sharded to sequence-sharded
    layout.  Core j owns query rows {512*qb + 64*j .. +64} for qb=0..3.
  - proj / residual / rmsnorm2 / FFN run sequence-sharded (256 rows/core)
    with replicated Wproj/W1/W2 streamed from HBM.
  - g1/g2 are folded into Wq/Wk/Wv/W1 on the host; bproj is added via a
    rank-1 matmul into the proj PSUM accumulation.
  - PE-facing tensors are fp16 (10-bit mantissa, ~4e-4 matmul rel err,
    full-rate matmul + fast weight load + half DMA); residual adds and
    softmax/norm statistics stay fp32.

Each core returns its 256 interleaved rows; the host scatters them back.
"""

import numpy as np
from contextlib import ExitStack

import concourse.bass as bass
import concourse.tile as tile
from concourse import bacc, mybir
from concourse import bass_utils

T, D, H, HS, FF = 2048, 1024, 16, 64, 4096
NCORES = 8
HPC = H // NCORES      # heads per core = 2
CH = T // NCORES       # rows per core = 256
QB = 512               # query block
NB = T // QB           # 4 query blocks
ND = D // 128          # 8 contraction tiles
NT = T // 128          # 16 key tiles
SUB = QB // NCORES     # 64 rows per (core, query block)
EPS = 1e-6
SCALE = HS ** -0.5

F32 = mybir.dt.float32
F16 = mybir.dt.float16
AF = mybir.ActivationFunctionType
ALU = mybir.AluOpType

_CACHE = {}


def build_nc():
    if "nc" in _CACHE:
        return _CACHE["nc"]

    nc = bacc.Bacc("TRN2", target_bir_lowering=False, debug=False, num_devices=NCORES)

    xT_d = nc.dram_tensor("xT", [D, T], F16, kind="ExternalInput")
    xch_d = nc.dram_tensor("xch", [CH, D], F32, kind="ExternalInput")
    wq_d = nc.dram_tensor("wq", [D, HPC * HS], F16, kind="ExternalInput")
    wk_d = nc.dram_tensor("wk", [D, HPC * HS], F16, kind="ExternalInput")
    wv_d = nc.dram_tensor("wv", [D, HPC * HS], F16, kind="ExternalInput")
    wp_d = nc.dram_tensor("wp", [D, D], F16, kind="ExternalInput")
    bp_d = nc.dram_tensor("bp", [1, D], F16, kind="ExternalInput")
    w1_d = nc.dram_tensor("w1", [D, FF], F16, kind="ExternalInput")
    w2_d = nc.dram_tensor("w2", [FF, D], F16, kind="ExternalInput")
    ident_d = nc.dram_tensor("ident", [128, 128], F16, kind="ExternalInput")
    ones_c_d = nc.dram_tensor("ones_c", [128, 1], F16, kind="ExternalInput")
    ones_r_d = nc.dram_tensor("ones_r", [1, 128], F16, kind="ExternalInput")
    onescol_d = nc.dram_tensor("onescol", [128, NT], F16, kind="ExternalInput")
    epsb_d = nc.dram_tensor("epsb", [128, 1], F32, kind="ExternalInput")
    out_d = nc.dram_tensor("out", [CH, D], F32, kind="ExternalOutput")

    with tile.TileContext(nc) as tc, ExitStack() as top:
        pers = top.enter_context(tc.tile_pool(name="pers", bufs=1))
        p2 = top.enter_context(tc.tile_pool(name="p2", bufs=2))
        dram = top.enter_context(tc.tile_pool(name="dram", bufs=1, space="DRAM"))

        ident = pers.tile([128, 128], F16, name="ident")
        nc.sync.dma_start(ident[:], ident_d[:])
        ones_c = pers.tile([128, 1], F16, name="ones_c")
        nc.sync.dma_start(ones_c[:], ones_c_d[:])
        ones_r = pers.tile([1, 128], F16, name="ones_r")
        nc.sync.dma_start(ones_r[:], ones_r_d[:])
        bp = pers.tile([1, D], F16, name="bp")
        nc.sync.dma_start(bp[:], bp_d[:])
        epsb = pers.tile([128, 1], F32, name="epsb")
        nc.sync.dma_start(epsb[:], epsb_d[:])

        qT = pers.tile([128, T], F16, name="qT")
        kT = pers.tile([128, T], F16, name="kT")
        va = [pers.tile([128, NT, HS + 1], F16, name=f"va{h}") for h in range(HPC)]
        for h in range(HPC):
            nc.sync.dma_start(va[h][:, :, HS], onescol_d[:])

        wp = [pers.tile([128, D], F16, name=f"wp{f}") for f in range(ND)]
        for f in range(ND):
            nc.sync.dma_start(wp[f][:], wp_d[128 * f : 128 * (f + 1), :])

        bnc_i = dram.tile([NCORES * 128, CH], F16, name="bnc_i")
        bnc_o = dram.tile([NCORES * 128, CH], F16, name="bnc_o")

        # ============ Phase A+B: rmsnorm1 (transposed layout) + QKV^T ============
        with ExitStack() as ph:
            pab = ph.enter_context(tc.tile_pool(name="pab", bufs=1))
            pstr = ph.enter_context(tc.tile_pool(name="pstr", bufs=3))
            psq = ph.enter_context(tc.tile_pool(name="psq", bufs=2, space="PSUM"))
            pss = ph.enter_context(tc.tile_pool(name="pss", bufs=1, space="PSUM"))
            ptp = ph.enter_context(tc.tile_pool(name="ptp", bufs=1, space="PSUM"))

            xt = [pab.tile([128, T], F16, name=f"xt{d}") for d in range(ND)]
            for tb in range(NB):
                cs = slice(QB * tb, QB * (tb + 1))
                for d in range(ND):
                    nc.scalar.dma_start(xt[d][:, cs], xT_d[128 * d : 128 * (d + 1), cs])
            wq = [pab.tile([128, HPC * HS], F16, name=f"wq{d}") for d in range(ND)]
            wk = [pab.tile([128, HPC * HS], F16, name=f"wk{d}") for d in range(ND)]
            wv = [pab.tile([128, HPC * HS], F16, name=f"wv{d}") for d in range(ND)]
            for d in range(ND):
                sl = slice(128 * d, 128 * (d + 1))
                nc.sync.dma_start(wq[d][:], wq_d[sl, :])
                nc.sync.dma_start(wk[d][:], wk_d[sl, :])
                nc.sync.dma_start(wv[d][:], wv_d[sl, :])

            # rmsnorm stats for every block first: squares on DVE, partition-sum
            # on PE, so nothing blocks the later dense QKV stream.
            bcs = []
            for tb in range(NB):
                cs = slice(QB * tb, QB * (tb + 1))
                ps_ss = pss.tile([1, QB], F32, name="ps_ss", tag="ps_ss")
                for d in range(ND):
                    sq = pstr.tile([128, QB], F16, name="sq")
                    nc.vector.tensor_mul(sq[:], xt[d][:, cs], xt[d][:, cs])
                    nc.tensor.matmul(
                        ps_ss[:], ones_c[:], sq[:], start=(d == 0), stop=(d == ND - 1)
                    )
                sqr = pstr.tile([1, QB], F32, name="sqr")
                nc.scalar.activation(
                    sqr[:], ps_ss[:], AF.Sqrt, scale=1.0 / D, bias=epsb[0:1, :]
                )
                rstd = pstr.tile([1, QB], F16, name="rstd")
                with nc.allow_low_precision("fp16 rounding is intended"):
                    nc.vector.reciprocal(rstd[:], sqr[:])
                bc = pab.tile([128, QB], F16, name=f"bc{tb}")
                nc.gpsimd.partition_broadcast(bc[:], rstd[:])
                bcs.append(bc)

            # raw QKV^T on unnormalized x; rstd is folded in at evacuation
            for tb in range(NB):
                cs = slice(QB * tb, QB * (tb + 1))
                bc = bcs[tb]
                ps_q = psq.tile([128, QB], F32, name="ps_q")
                ps_k = psq.tile([128, QB], F32, name="ps_k")
                ps_v = psq.tile([128, QB], F32, name="ps_v")
                for d in range(ND):
                    st, sp = (d == 0), (d == ND - 1)
                    nc.tensor.matmul(ps_q[:], wq[d][:], xt[d][:, cs], start=st, stop=sp)
                    nc.tensor.matmul(ps_k[:], wk[d][:], xt[d][:, cs], start=st, stop=sp)
                    nc.tensor.matmul(ps_v[:], wv[d][:], xt[d][:, cs], start=st, stop=sp)
                nc.vector.tensor_mul(qT[:, cs], ps_q[:], bc[:])
                nc.vector.tensor_mul(kT[:, cs], ps_k[:], bc[:])
                vt = pstr.tile([128, QB], F16, name="vt")
                nc.vector.tensor_mul(vt[:], ps_v[:], bc[:])
                for s in range(4):
                    tt = 4 * tb + s
                    ps_t = ptp.tile([128, 128], F16, name="ps_t")
                    nc.tensor.transpose(
                        ps_t[:], vt[:, 128 * s : 128 * (s + 1)], ident[:]
                    )
                    for h in range(HPC):
                        nc.vector.tensor_copy(
                            va[h][:, tt, 0:HS], ps_t[:, HS * h : HS * (h + 1)]
                        )

        # ============ Phase C: causal attention + per-block AllToAll =============
        with ExitStack() as ph:
            ppt = ph.enter_context(tc.tile_pool(name="ppt", bufs=4))
            psat = ph.enter_context(tc.tile_pool(name="psat", bufs=2, space="PSUM"))
            pscs = ph.enter_context(tc.tile_pool(name="pscs", bufs=2, space="PSUM"))

            for qb in range(NB):
                qs = slice(QB * qb, QB * (qb + 1))
                nkt = 4 * (qb + 1)
                ps_at = [
                    psat.tile([HS + 1, QB], F32, name=f"at{h}", tag=f"at{h}")
                    for h in range(HPC)
                ]
                for kt0 in range(0, nkt, 2):
                    for h in range(HPC):
                        hsl = slice(HS * h, HS * (h + 1))
                        ps_s = pscs.tile([128, 1024], F32, name="ps_s", tag="ps_s")
                        for i in range(2):
                            kt = kt0 + i
                            nc.tensor.matmul(
                                ps_s[:, 512 * i : 512 * (i + 1)],
                                kT[hsl, 128 * kt : 128 * (kt + 1)],
                                qT[hsl, qs],
                                start=True,
                                stop=True,
                            )
                        pt = ppt.tile([128, 1024], F16, name="pt")
                        nc.scalar.activation(pt[:], ps_s[:], AF.Exp, scale=SCALE)
                        for i in range(2):
                            kt = kt0 + i
                            if kt >= 4 * qb:  # diagonal tile: zero where k > q
                                nc.gpsimd.affine_select(
                                    pt[:, 512 * i : 512 * (i + 1)],
                                    pt[:, 512 * i : 512 * (i + 1)],
                                    pattern=[[1, 512]],
                                    compare_op=ALU.is_ge,
                                    fill=0.0,
                                    base=QB * qb - 128 * kt,
                                    channel_multiplier=-1,
                                )
                        for i in range(2):
                            kt = kt0 + i
                            nc.tensor.matmul(
                                ps_at[h][:],
                                va[h][:, kt, :],
                                pt[:, 512 * i : 512 * (i + 1)],
                                start=(kt == 0),
                                stop=(kt == nkt - 1),
                            )
                attnT2 = p2.tile([128, QB], F16, name="attnT2")
                for h in range(HPC):
                    hsl = slice(HS * h, HS * (h + 1))
                    recip_h = p2.tile([1, QB], F16, name="recip", tag="recip")
                    with nc.allow_low_precision("fp16 rounding is intended"):
                        nc.vector.reciprocal(recip_h[:], ps_at[h][HS : HS + 1, :])
                    bc_sb = p2.tile([HS, QB], F16, name="bc_sb", tag="bc_sb")
                    nc.gpsimd.partition_broadcast(bc_sb[:], recip_h[:])
                    nc.vector.tensor_mul(attnT2[hsl, :], ps_at[h][0:HS, :], bc_sb[:])
                for half in range(2):
                    j = 2 * qb + half
                    nc.scalar.dma_start(
                        bnc_i[128 * j : 128 * (j + 1), :],
                        attnT2[:, 256 * half : 256 * (half + 1)],
                    )

            nc.gpsimd.collective_compute(
                "AllToAll",
                ALU.bypass,
                replica_groups=[list(range(NCORES))],
                ins=[bnc_i.opt()],
                outs=[bnc_o.opt()],
            )

        # ============ Phase D: proj + residual + rmsnorm2 (+ transpose) ==========
        x2 = [pers.tile([128, D], F32, name=f"x2_{ts}") for ts in range(2)]
        xn2T = [pers.tile([128, CH], F16, name=f"xn2T{d}") for d in range(ND)]
        with ExitStack() as ph:
            pd = ph.enter_context(tc.tile_pool(name="pd", bufs=1))
            pds = ph.enter_context(tc.tile_pool(name="pds", bufs=2))
            psd1 = ph.enter_context(tc.tile_pool(name="psd1", bufs=2, space="PSUM"))
            psd2 = ph.enter_context(tc.tile_pool(name="psd2", bufs=2, space="PSUM"))

            aT = [pd.tile([128, CH], F16, name=f"aT{f}") for f in range(ND)]
            for f in range(ND):
                nc.scalar.dma_start(aT[f][:], bnc_o[128 * f : 128 * (f + 1), :])
            xch = [pd.tile([128, D], F32, name=f"xch{ts}") for ts in range(2)]
            for ts in range(2):
                nc.scalar.dma_start(xch[ts][:], xch_d[128 * ts : 128 * (ts + 1), :])

            for ts in range(2):
                tsl = slice(128 * ts, 128 * (ts + 1))
                ps_sa = psd1.tile([128, D], F32, name="ps_sa")
                for b in range(2):
                    bsl = slice(512 * b, 512 * (b + 1))
                    nc.tensor.matmul(
                        ps_sa[:, bsl], ones_r[:], bp[0:1, bsl], start=True, stop=False
                    )
                    for f in range(ND):
                        nc.tensor.matmul(
                            ps_sa[:, bsl],
                            aT[f][:, tsl],
                            wp[f][:, bsl],
                            start=False,
                            stop=(f == ND - 1),
                        )
                nc.vector.tensor_add(x2[ts][:], ps_sa[:], xch[ts][:])
                sq2 = pds.tile([128, D], F32, name="sq2")
                ss2 = pds.tile([128, 1], F32, name="ss2")
                nc.scalar.activation(sq2[:], x2[ts][:], AF.Square, accum_out=ss2[:])
                sqr2 = pds.tile([128, 1], F32, name="sqr2")
                nc.scalar.activation(
                    sqr2[:], ss2[:], AF.Sqrt, scale=1.0 / D, bias=epsb[:]
                )
                rstd2 = pds.tile([128, 1], F32, name="rstd2")
                nc.vector.reciprocal(rstd2[:], sqr2[:])
                xn2 = pds.tile([128, D], F16, name="xn2")
                with nc.allow_low_precision("fp16 rounding is intended"):
                    nc.scalar.activation(xn2[:], x2[ts][:], AF.Copy, scale=rstd2[:])
                for d in range(ND):
                    ps_t2 = psd2.tile([128, 128], F16, name="ps_t2")
                    nc.tensor.transpose(
                        ps_t2[:], xn2[:, 128 * d : 128 * (d + 1)], ident[:]
                    )
                    nc.vector.tensor_copy(xn2T[d][:, tsl], ps_t2[:])

        # ============ Phase E: FFN (replicated fp16 weights, 256 rows) ===========
        with ExitStack() as ph:
            pw1 = ph.enter_context(tc.tile_pool(name="pw1", bufs=16))
            pw2 = ph.enter_context(tc.tile_pool(name="pw2", bufs=6))
            pht = ph.enter_context(tc.tile_pool(name="pht", bufs=4))
            pse1 = ph.enter_context(tc.tile_pool(name="pse1", bufs=1, space="PSUM"))
            pse3 = ph.enter_context(tc.tile_pool(name="pse3", bufs=3, space="PSUM"))

            ps_out = [pse1.tile([128, D], F32, name=f"ps_out{ts}") for ts in range(2)]
            NG = 8
            for g in range(NG):
                gsl = slice(512 * g, 512 * (g + 1))
                w1g = []
                for d in range(ND):
                    t_ = pw1.tile([128, 512], F16, name="w1t", tag="w1t")
                    nc.sync.dma_start(t_[:], w1_d[128 * d : 128 * (d + 1), gsl])
                    w1g.append(t_)
                for j in range(4):
                    ff = 4 * g + j
                    ps_h = pse3.tile([128, CH], F32, name="ps_h")
                    for d in range(ND):
                        nc.tensor.matmul(
                            ps_h[:],
                            w1g[d][:, 128 * j : 128 * (j + 1)],
                            xn2T[d][:],
                            start=(d == 0),
                            stop=(d == ND - 1),
                        )
                    hT = pht.tile([128, CH], F16, name="hT", tag="hT")
                    nc.scalar.activation(hT[:], ps_h[:], AF.Silu)
                    w2t = pw2.tile([128, D], F16, name="w2t", tag="w2t")
                    nc.sync.dma_start(w2t[:], w2_d[128 * ff : 128 * (ff + 1), :])
                    for ts in range(2):
                        for b in range(2):
                            nc.tensor.matmul(
                                ps_out[ts][:, 512 * b : 512 * (b + 1)],
                                hT[:, 128 * ts : 128 * (ts + 1)],
                                w2t[:, 512 * b : 512 * (b + 1)],
                                start=(ff == 0),
                                stop=(ff == FF // 128 - 1),
                            )
            for ts in range(2):
                out_t = pht.tile([128, D], F32, name=f"out{ts}", tag=f"out{ts}")
                nc.vector.tensor_add(out_t[:], ps_out[ts][:], x2[ts][:])
                nc.scalar.dma_start(out_d[128 * ts : 128 * (ts + 1), :], out_t[:])

    nc.compile()
    _CACHE["nc"] = nc
    return nc


def _chunk_rows(j):
    """Row indices owned by core j (contiguous 256-row chunk)."""
    return np.arange(CH * j, CH * (j + 1))


def make_in_maps(inputs):
    x = np.asarray(inputs["x"], np.float32).reshape(T, D)
    Wq = np.asarray(inputs["Wq"], np.float32)
    Wk = np.asarray(inputs["Wk"], np.float32)
    Wv = np.asarray(inputs["Wv"], np.float32)
    Wproj = np.asarray(inputs["Wproj"], np.float32)
    bproj = np.asarray(inputs["bproj"], np.float32).reshape(1, D)
    W1 = np.asarray(inputs["W1"], np.float32)
    W2 = np.asarray(inputs["W2"], np.float32)
    g1 = np.asarray(inputs["g1"], np.float32)
    g2 = np.asarray(inputs["g2"], np.float32)

    xT = np.ascontiguousarray(x.T).astype(np.float16)
    Wq_f = (Wq * g1[None, :, None]).astype(np.float16)
    Wk_f = (Wk * g1[None, :, None]).astype(np.float16)
    Wv_f = (Wv * g1[None, :, None]).astype(np.float16)
    W1_f = np.ascontiguousarray(W1 * g2[:, None]).astype(np.float16)
    W2_h = np.ascontiguousarray(W2.astype(np.float16))
    Wp_h = np.ascontiguousarray(Wproj.astype(np.float16))

    common = {
        "xT": xT,
        "wp": Wp_h,
        "bp": bproj.astype(np.float16),
        "w1": W1_f,
        "w2": W2_h,
        "ident": np.eye(128, dtype=np.float16),
        "ones_c": np.ones((128, 1), np.float16),
        "ones_r": np.ones((1, 128), np.float16),
        "onescol": np.ones((128, NT), np.float16),
        "epsb": np.full((128, 1), EPS, np.float32),
    }
    in_maps = []
    for c in range(NCORES):
        heads = [HPC * c + h for h in range(HPC)]
        in_maps.append(
            {
                **common,
                "xch": np.ascontiguousarray(x[_chunk_rows(c)]),
                "wq": np.ascontiguousarray(np.concatenate([Wq_f[h] for h in heads], 1)),
                "wk": np.ascontiguousarray(np.concatenate([Wk_f[h] for h in heads], 1)),
                "wv": np.ascontiguousarray(np.concatenate([Wv_f[h] for h in heads], 1)),
            }
        )
    return in_maps


def run(inputs, **kwargs):
    nc = build_nc()
    in_maps = make_in_maps(inputs)
    res = bass_utils.run_bass_kernel_spmd(
        nc, in_maps, core_ids=list(range(NCORES)), **kwargs
    )
    out = np.empty((T, D), np.float32)
    for c in range(NCORES):
        out[_chunk_rows(c)] = res.results[c]["out"]
    return out.reshape(1, T, D), res


def kernel(**inputs):
    out, _ = run(inputs)
    return out
